# revision 1
# baseline (speedup 1.0000x reference)
"""Nearest-neighbor tokenizer on Trainium2: 8 NeuronCores, code-sharded.

Per token x (d=512) against codebook C [16384, 512]:
    dist^2(x,c) = ||x||^2 + ||c||^2 - 2 x.c
    id = argmin_c dist^2   if min_c dist^2 <= 900 else -1

v2 architecture (candidate search on device, exact rescore on host):
  - Shard by CODES: core g owns codes[g*2048:(g+1)*2048] and sees all
    8192 tokens (64 token tiles of 128).
  - Device computes v_c = x.c - ||c||^2/2 in ONE fp32r GEMM pass per
    tile. The -||c||^2/2 bias rides as a K=2 matmul (hi/lo split of the
    bias, hi exactly representable in f32r) that opens each PSUM
    accumulation group, so the GEMM result lands pre-biased in PSUM.
  - DVE pair-maxes the 2048 v values into 1024 (one PSUM + one
    ACT-drained SBUF operand), then top-8 + indices per token.
  - Host merges 8 cores x 8 pairs x 2 codes = 128 candidates/token and
    rescores them exactly in float64; argmin + threshold reproduce the
    reference bit-exactly as long as the true winner is among the
    candidates (fp32r noise ~2e-3 vs needing 8 closer pairs: safe).
"""

import sys

import numpy as np

try:
    import concourse.bass as _probe_bass  # noqa: F401
except Exception:  # pragma: no cover
    sys.path.insert(0, "/opt/trn_rl_repo")

B, S, D = 4, 2048, 512
C = 16384
N_CORES = 8
NTOK = B * S                   # 8192 tokens, all seen by every core
N_TILES = NTOK // 128          # 64 token tiles
G = C // N_CORES               # 2048 codes per core
KC = D // 128                  # 4 contraction chunks
NSLC = G // 512                # 4 psum bank slices
HALF = G // 2                  # 1024 pairs

_CACHE: dict = {}


def _build_program(nc=None):
    import concourse.tile as tile
    from concourse import mybir

    f32 = mybir.dt.float32
    f32r = mybir.dt.float32r
    u32 = mybir.dt.uint32
    Alu = mybir.AluOpType
    Act = mybir.ActivationFunctionType

    if nc is None:
        # Bacc: its finalize() runs the TRN2 wait-splitting compile passes
        # (plain Bass emits multi-wait DMAs that walrus codegen rejects).
        from concourse import bacc

        nc = bacc.Bacc("TRN2", target_bir_lowering=False, debug=False)

    xs_d = nc.declare_dram_parameter("xs", [128, N_TILES * D], f32, isOutput=False)
    cr_d = nc.declare_dram_parameter("cr", [128, KC * G], f32, isOutput=False)
    cb2_d = nc.declare_dram_parameter("cb2", [2, G], f32, isOutput=False)
    cval_d = nc.declare_dram_parameter("cval", [128, N_TILES * 8], f32, isOutput=True)
    cidx_d = nc.declare_dram_parameter("cidx", [128, N_TILES * 8], u32, isOutput=True)

    with tile.TileContext(nc) as tc:
        with (
            tc.tile_pool(name="const", bufs=1) as const,
            tc.tile_pool(name="work", bufs=3) as work,
            tc.tile_pool(name="psum", bufs=2, space="PSUM") as psum,
        ):
            # One-time: codes + bias to SBUF, rounded to f32r.
            crb = const.tile([128, KC * G], f32, name="crb")
            nc.sync.dma_start(crb[:], cr_d[:])
            crr = const.tile([128, KC * G], f32r, name="crr")
            nc.vector.tensor_copy(crr[:], crb[:])
            cb2b = const.tile([2, G], f32, name="cb2b")
            nc.sync.dma_start(cb2b[:], cb2_d[:])
            cb2r = const.tile([2, G], f32r, name="cb2r")
            nc.vector.tensor_copy(cb2r[:], cb2b[:])
            onesb = const.tile([2, 128], f32, name="onesb")
            nc.vector.memset(onesb[:], 1.0)
            onesr = const.tile([2, 128], f32r, name="onesr")
            nc.vector.tensor_copy(onesr[:], onesb[:])

            cval = const.tile([128, N_TILES * 8], f32, name="cval")
            cidx = const.tile([128, N_TILES * 8], u32, name="cidx")

            for t in range(N_TILES):
                xsb = work.tile([128, D], f32, name="xsb")
                nc.sync.dma_start(xsb[:], xs_d[:, t * D:(t + 1) * D])
                xr = work.tile([128, D], f32r, name="xr")
                nc.scalar.activation(xr[:], xsb[:], Act.Copy)

                ps = psum.tile([128, G], f32, name="ps")
                for s in range(NSLC):
                    nc.tensor.matmul(
                        ps[:, s * 512:(s + 1) * 512],
                        onesr[:],
                        cb2r[:, s * 512:(s + 1) * 512],
                        start=True,
                        stop=False,
                    )
                    for k in range(KC):
                        nc.tensor.matmul(
                            ps[:, s * 512:(s + 1) * 512],
                            xr[:, k * 128:(k + 1) * 128],
                            crr[:, k * G + s * 512:k * G + (s + 1) * 512],
                            start=False,
                            stop=(k == KC - 1),
                        )

                h1 = work.tile([128, HALF], f32, name="h1")
                nc.scalar.activation(h1[:], ps[:, HALF:], Act.Copy)
                pm = work.tile([128, HALF], f32, name="pm")
                nc.vector.tensor_tensor(pm[:], ps[:, :HALF], h1[:], Alu.max)
                nc.vector.max(cval[:, t * 8:(t + 1) * 8], pm[:])
                nc.vector.max_index(
                    cidx[:, t * 8:(t + 1) * 8], cval[:, t * 8:(t + 1) * 8], pm[:]
                )

            nc.sync.dma_start(cval_d[:], cval[:])
            nc.sync.dma_start(cidx_d[:], cidx[:])

    return nc


def _prepare_in_maps(x: np.ndarray, codes: np.ndarray) -> list:
    x = np.ascontiguousarray(np.asarray(x, dtype=np.float32).reshape(NTOK, D))
    codes = np.ascontiguousarray(np.asarray(codes, dtype=np.float32))

    # xs[p, t*512 + k*128 + m] = x[t*128 + m, k*128 + p]  (same for all cores)
    xs = np.ascontiguousarray(
        x.reshape(N_TILES, 128, KC, 128).transpose(3, 0, 2, 1).reshape(128, -1)
    )

    in_maps = []
    for g in range(N_CORES):
        cg = codes[g * G:(g + 1) * G]  # [2048, 512]
        # cr[p, k*2048 + n] = cg[n, k*128 + p]
        cr = np.ascontiguousarray(
            cg.reshape(G, KC, 128).transpose(2, 1, 0).reshape(128, -1)
        )
        c2neg = (-0.5 * (cg.astype(np.float64) ** 2).sum(1)).astype(np.float32)
        # hi: keep top 11 mantissa bits -> exactly representable in f32r,
        # so the on-device f32r rounding of hi is the identity.
        hi = (c2neg.view(np.uint32) & np.uint32(0xFFFFF000)).view(np.float32)
        lo = (c2neg.astype(np.float64) - hi).astype(np.float32)
        cb2 = np.ascontiguousarray(np.stack([hi, lo]).astype(np.float32))
        in_maps.append({"xs": xs, "cr": cr, "cb2": cb2})
    return in_maps


def _postprocess(results: list, x: np.ndarray, codes: np.ndarray) -> np.ndarray:
    x64 = np.asarray(x, dtype=np.float64).reshape(NTOK, D)
    c64 = np.asarray(codes, dtype=np.float64)
    c2 = (c64 ** 2).sum(1)
    x2 = (x64 ** 2).sum(1)

    # cidx[g]: [128, 64*8]; token = t*128 + partition; local pair j -> codes
    # {g*2048 + j, g*2048 + j + 1024}.
    cand = np.empty((NTOK, N_CORES * 8), np.int64)
    for g in range(N_CORES):
        ci = np.asarray(results[g]["cidx"]).astype(np.int64)
        ci = ci.reshape(128, N_TILES, 8).transpose(1, 0, 2).reshape(NTOK, 8)
        cand[:, g * 8:(g + 1) * 8] = ci + g * G
    cands = np.concatenate([cand, cand + HALF], axis=1)  # [NTOK, 128]
    cands.sort(axis=1)  # argmin tie-break: first occurrence = lowest index

    ids = np.empty(NTOK, np.int64)
    CH = 1024
    rows = np.arange(CH)
    for i in range(0, NTOK, CH):
        cc = cands[i:i + CH]
        xc = np.einsum("tkd,td->tk", c64[cc], x64[i:i + CH], optimize=True)
        d2 = np.maximum(x2[i:i + CH, None] + c2[cc] - 2.0 * xc, 0.0)
        k = d2.argmin(1)
        ids[i:i + CH] = np.where(d2[rows, k] <= 900.0, cc[rows, k], -1)
    return ids.reshape(B, S).astype(np.int32)


def kernel(x: np.ndarray, codes: np.ndarray) -> np.ndarray:
    from concourse.bass_utils import run_bass_kernel_spmd

    if "nc" not in _CACHE:
        nc = _build_program()
        nc.finalize()  # Bacc: runs wait-splitting + register allocation
        _CACHE["nc"] = nc
    in_maps = _prepare_in_maps(x, codes)
    res = run_bass_kernel_spmd(_CACHE["nc"], in_maps, list(range(N_CORES)))
    return _postprocess(res.results, x, codes)



# revision 39
# speedup vs baseline: 3.6410x; 3.6410x over previous
"""Nearest-neighbor tokenizer on Trainium2: 8 NeuronCores, code-sharded.

Per token x (d=512) against codebook C [16384, 512]:
    dist^2(x,c) = ||x||^2 + ||c||^2 - 2 x.c
    id = argmin_c dist^2   if min_c dist^2 <= 900 else -1

v4 architecture (fp8 DoubleRow GEMM + window partial-max on device,
exact branch-and-bound rescore on host):
  - Codes are sorted by ||c||^2 on the host; core g owns sorted codes
    [g*2048, (g+1)*2048) and sees all 8192 tokens (64 tiles of 128).
  - Device computes ONLY the raw inner products v = x.c in fp8(e4m3)
    with DoubleRow matmuls (K=256 per call, 0.5 PE cycles/row): per
    512-code PSUM slice just 2 matmuls instead of f32r's 4+bias. The
    tensor engine is the roofline at ~853 ns/tile.
  - Drain: real TRN2 restricts this hard (BIR verifier): GPSIMD
    cannot touch PSUM, no instruction may read TWO PSUM operands,
    and GPSIMD has no two-tensor ALU ops at all. So the only PSUM
    drains are ACT (pointwise convert, 0.833 ns/elem) and DVE
    (windowed tensor_reduce, single PSUM input, 1.04 ns/elem):
      ACT  converts windows 0-15 (bank-aligned ps_a) to fp16 and
           writes them STRAIGHT into the staging output tile,
      DVE  collapses windows 16-31 (ps_b) to their maxima.
    ACT-side windows ship raw width-64 fp16 ([128, 64, 16, 64]; the
    host takes the window max); DVE-side windows ship exact fp16
    maxima ([128, 64, 16]). ps_a is read only by ACT and ps_b only
    by DVE, so the PSUM free chains stay decoupled.
  - Host: windows are norm-homogeneous (codes sorted), so
    [devmax_w - bmax_w - M, devmax_w - bmin_w + M] brackets the best
    biased score v - ||c||^2/2 of window w (M covers fp8+fp16 noise,
    measured absmax 3.8 on-device). Rescore every window whose upper
    bound reaches the best lower bound, exactly, in float64; argmin +
    threshold then reproduce the reference bit-exactly as long as the
    true winner's window is rescored (margin M=6.5).
"""

import sys

import numpy as np

try:
    import concourse.bass as _probe_bass  # noqa: F401
except Exception:  # pragma: no cover
    sys.path.insert(0, "/opt/trn_rl_repo")

B, S, D = 4, 2048, 512
C = 16384
N_CORES = 8
NTOK = B * S                   # 8192 tokens, all seen by every core
N_TILES = NTOK // 128          # 64 token tiles
G = C // N_CORES               # 2048 codes per core
W = 64                         # candidate window size (codes)
NWC = G // W                   # 32 windows per core
ACT_NW = 16                    # windows drained via ACT fp16 convert
DVE_NW = NWC - ACT_NW          # 16 windows DVE reduces straight from PSUM
WOUT = W                       # ACT-side windows ship raw width-64 values
MARGIN = 6.5                   # abs bound on fp8 GEMM + fp16 rounding noise

_CACHE: dict = {}


def _build_program(nc=None):
    import concourse.tile as tile
    from concourse import mybir

    f32 = mybir.dt.float32
    f16 = mybir.dt.float16
    fp8 = mybir.dt.float8e4
    Alu = mybir.AluOpType
    Act = mybir.ActivationFunctionType

    if nc is None:
        # Bacc: its finalize() runs the TRN2 wait-splitting compile passes
        # (plain Bass emits multi-wait DMAs that walrus codegen rejects).
        from concourse import bacc

        nc = bacc.Bacc("TRN2", target_bir_lowering=False, debug=False)

    # xq[p, t, kg, i, m] = fp8(x)[t*128+m, kg*256+i*128+p]
    xq_d = nc.declare_dram_parameter("xq", [128, N_TILES, 2, 2, 128], fp8,
                                     isOutput=False)
    # cq[p, kg, i, n] = fp8(c_sorted_core)[n, kg*256+i*128+p]
    cq_d = nc.declare_dram_parameter("cq", [128, 2, 2, G], fp8, isOutput=False)
    wmax_d = nc.declare_dram_parameter("wmax", [128, N_TILES, ACT_NW, WOUT],
                                       f16, isOutput=True)
    wdve_d = nc.declare_dram_parameter("wdve", [128, N_TILES, DVE_NW], f16,
                                       isOutput=True)

    dr = mybir.MatmulPerfMode.DoubleRow

    with tile.TileContext(nc) as tc:
        with (
            tc.tile_pool(name="const", bufs=1) as const,
            tc.tile_pool(name="psum", bufs=2, space="PSUM") as psum,
        ):
            # Startup DMAs spread over the Pool and SP queues so descriptor
            # generation and the transfers themselves run in parallel. (Not
            # the ACT queue: it is busy with its act-table load.)
            cq = const.tile([128, 2, 2, G], fp8, name="cq")
            for kg in range(2):
                nc.gpsimd.dma_start(cq[:, kg, 0], cq_d[:, kg, 0])
                nc.sync.dma_start(cq[:, kg, 1], cq_d[:, kg, 1])
            xq = const.tile([128, N_TILES, 2, 2, 128], fp8, name="xq")
            xch = [(0, 2), (2, 4)] + [(c, c + 4) for c in range(4, N_TILES, 4)]
            for lo, hi in xch:
                nc.sync.dma_start(xq[:, lo:hi], xq_d[:, lo:hi])

            wmax = const.tile([128, N_TILES, ACT_NW, WOUT], f16, name="wmax")
            wdve = const.tile([128, N_TILES, DVE_NW], f16, name="wdve")

            for t in range(N_TILES):
                # Bank-aligned single-consumer PSUM tiles: ps_a (2 banks)
                # is read only by ACT, ps_b (2 banks) only by DVE. No
                # cross-engine coupling on the free chains.
                ps_a = psum.tile([128, ACT_NW, W], f32, name="ps_a")
                ps_b = psum.tile([128, DVE_NW, W], f32, name="ps_b")

                def mm(s):
                    half = ps_a if s < 2 else ps_b
                    so = s % 2
                    for kg in range(2):
                        nc.tensor.matmul(
                            half[:, so * 8:(so + 1) * 8],
                            xq[:, t, kg],
                            cq[:, kg, :, s * 512:(s + 1) * 512],
                            start=(kg == 0),
                            stop=(kg == 1),
                            perf_mode=dr,
                        )

                mm(0)
                mm(1)
                # ACT: windows 0-15 -> fp16, straight into the staging
                # output tile, parallel to the ps_b matmuls below.
                nc.scalar.activation(wmax[:, t], ps_a[:], Act.Copy)
                mm(2)
                mm(3)
                # DVE: windowed max of ps_b (windows 16-31), one reduce.
                nc.vector.tensor_reduce(
                    wdve[:, t], ps_b[:], axis=mybir.AxisListType.X,
                    op=Alu.max,
                )

                # Output DMA: 4-tile chunks, but per-tile near the end so
                # only the very last tile's small DMA sits in the tail.
                if t >= N_TILES - 4:
                    nc.sync.dma_start(wmax_d[:, t:t + 1], wmax[:, t:t + 1])
                elif t % 4 == 3:
                    nc.sync.dma_start(
                        wmax_d[:, t - 3:t + 1], wmax[:, t - 3:t + 1]
                    )
                if t % 16 == 15:
                    nc.sync.dma_start(
                        wdve_d[:, t - 15:t + 1], wdve[:, t - 15:t + 1]
                    )

    return nc


def _prepare(x: np.ndarray, codes: np.ndarray):
    import ml_dtypes

    e4 = ml_dtypes.float8_e4m3
    x = np.ascontiguousarray(np.asarray(x, dtype=np.float32).reshape(NTOK, D))
    codes = np.ascontiguousarray(np.asarray(codes, dtype=np.float32))

    c2 = (codes.astype(np.float64) ** 2).sum(1)
    order = np.argsort(c2, kind="stable")
    cs = codes[order]

    x8 = x.astype(e4)
    cs8 = cs.astype(e4)

    # xq[p, t, kg, i, m] = x8[t*128+m, kg*256+i*128+p]
    xq = np.ascontiguousarray(
        x8.reshape(N_TILES, 128, 2, 2, 128).transpose(4, 0, 2, 3, 1)
    )
    in_maps = []
    for g in range(N_CORES):
        cg8 = cs8[g * G:(g + 1) * G]
        # cq[p, kg, i, n] = cg8[n, kg*256+i*128+p]
        cq = np.ascontiguousarray(
            cg8.reshape(G, 2, 2, 128).transpose(3, 1, 2, 0)
        )
        in_maps.append({"xq": xq, "cq": cq})
    return in_maps, order, cs, c2[order]


def _postprocess(results, x, order, cs, c2s) -> np.ndarray:
    x64 = np.asarray(x, dtype=np.float64).reshape(NTOK, D)
    x2 = (x64 ** 2).sum(1)

    NW = C // W                                   # 256 global windows
    devmax = np.empty((NTOK, NW), np.float32)
    for g in range(N_CORES):
        wm = np.asarray(results[g]["wmax"]).astype(np.float32)
        wm = wm.reshape(128, N_TILES, ACT_NW, WOUT).max(3)
        wd = np.asarray(results[g]["wdve"]).astype(np.float32)
        wd = wd.reshape(128, N_TILES, DVE_NW)
        full = np.concatenate([wm, wd], axis=2)           # [128, 64, 32]
        devmax[:, g * NWC:(g + 1) * NWC] = (
            full.transpose(1, 0, 2).reshape(NTOK, NWC)
        )

    b = 0.5 * c2s                                 # bias per sorted code
    bwin = b.reshape(NW, W)
    bmin = bwin.min(1)
    bmax = bwin.max(1)

    dm = devmax.astype(np.float64)
    upper = dm - bmin[None, :] + MARGIN
    lower = dm - bmax[None, :] - MARGIN
    L = lower.max(1)
    mask = upper >= L[:, None]                    # windows to rescore

    best_d2 = np.full(NTOK, np.inf)
    best_id = np.full(NTOK, -1, np.int64)
    cs64 = cs.astype(np.float64)
    for w in range(NW):
        tok = np.nonzero(mask[:, w])[0]
        if tok.size == 0:
            continue
        cw = cs64[w * W:(w + 1) * W]              # [W, 512]
        ids_w = order[w * W:(w + 1) * W]          # original code indices
        d2w = np.maximum(
            x2[tok, None] + (cw ** 2).sum(1)[None, :]
            - 2.0 * (x64[tok] @ cw.T), 0.0,
        )
        jmin = d2w.argmin(1)
        rows = np.arange(tok.size)
        dmin = d2w[rows, jmin]
        imin = ids_w[jmin]
        upd = (dmin < best_d2[tok]) | (
            (dmin == best_d2[tok]) & (imin < best_id[tok])
        )
        ut = tok[upd]
        best_d2[ut] = dmin[upd]
        best_id[ut] = imin[upd]

    ids = np.where(best_d2 <= 900.0, best_id, -1)
    return ids.reshape(B, S).astype(np.int32)


def kernel(x: np.ndarray, codes: np.ndarray) -> np.ndarray:
    from concourse.bass_utils import run_bass_kernel_spmd

    if "nc" not in _CACHE:
        nc = _build_program()
        nc.finalize()  # Bacc: runs wait-splitting + register allocation
        _CACHE["nc"] = nc
    in_maps, order, cs, c2s = _prepare(x, codes)
    res = run_bass_kernel_spmd(_CACHE["nc"], in_maps, list(range(N_CORES)))
    return _postprocess(res.results, x, order, cs, c2s)
